# revision 20
# baseline (speedup 1.0000x reference)
"""Trainium2 Bass kernel for nn_CryptoGNN (2-layer GCN + pooled heads).

Math (same collapse as validated baseline):
  With A = normalized adjacency (incl. self loops), P = [B,N] pooling,
  u[d] = sum_{s->d} dis[s]x[s] + dis[d]x[d],  zhat = u@W1 + sq*b1,
  h1 = dis*relu(zhat) = dis*relu(zhat);  relu commutes with dis>0 so
  G = [PA;P]^T-pooled h1 uses papt columns pre-scaled by dis and
  h1hat = relu(zhat) on device.  Layer 2 + heads collapse to the host
  (tiny [64,*] math).

Per-core device pipeline (8-way node sharding, 12544 dst nodes/core):
  1. bf16 compacted src table DMA ([128, TW]: 8 banks x 6 feature rows,
     only srcs with >=1 edge into this core; all 128 rows host-written
     so no SBUF garbage can reach the PE - 0*NaN != 0 on TRN2 PE)
  2. expand bf16->fp32 split across Act/DVE (gather needs 4B elems)
  3. one GPSIMD ap_gather of all dst-sorted per-bank edge streams
  4. per dst-chunk (7 chunks x 1792 dsts): fp32 prefix scan (DVE),
     boundary ap_gather (GPSIMD), shifted diff -> dt bf16 (DVE 2x mode)
  5. per node tile: z = dt_t^T @ selW + aug_t^T @ w1aug   (bf16 PE,
     selW = bank-scattered W1 rows; fold+mm1 fused, no PSUM->SBUF hop)
  6. relu -> h1 bf16 (Act), G^T += h1_t^T @ papt_t accumulated in one
     [128, 80] PSUM across all 98 tiles; single gout DMA.
Host sums the 8 partial G^T and runs the small head in numpy.
"""

import sys

if "/opt/trn_rl_repo" not in sys.path:
    sys.path.insert(0, "/opt/trn_rl_repo")

import numpy as np
import ml_dtypes

N = 100000
E = 600000
B = 64
IN = 6
H = 128
S = 16

NG = 8                    # banks (= src chunks) and cores
NS = 12544                # nodes per core shard (98*128)
NPAD = NS * NG            # 100352
NT = 98                   # node tiles per shard
# dst chunks per core, in node tiles; last chunk small to shrink the tail
TCH = (15, 15, 15, 15, 15, 15, 8)
C = len(TCH)
NDCS = tuple(t * 128 for t in TCH)            # dsts per chunk
# boundary widths: +32 keeps int16 idx slice offsets 4-byte aligned
NBCS = tuple(n + 32 for n in NDCS)
DOFF = tuple(int(x) for x in np.concatenate([[0], np.cumsum(NDCS)]))
BOFF = tuple(int(x) for x in np.concatenate([[0], np.cumsum(NBCS)]))
NBT = BOFF[-1]
PCOL = 80                 # papt columns: 64 PA + <=16 local P
P128 = 128

_compiled = {}


def _build_nc(TW, JWS):
    import concourse.bacc as bacc
    import concourse.mybir as mybir
    from concourse import tile

    f32 = mybir.dt.float32
    bf16 = mybir.dt.bfloat16
    i16 = mybir.dt.int16

    JWT = sum(JWS)
    OFF = np.concatenate([[0], np.cumsum(JWS)]).astype(int)

    nc = bacc.Bacc("TRN2", target_bir_lowering=False, debug=False)

    xt = nc.declare_dram_parameter("xt", [P128, TW], bf16, isOutput=False)
    gidx = nc.declare_dram_parameter("gidx", [P128, JWT // 16], i16, isOutput=False)
    bidx = nc.declare_dram_parameter("bidx", [P128, NBT // 16], i16, isOutput=False)
    selw = nc.declare_dram_parameter("selw", [P128, H], bf16, isOutput=False)
    papt = nc.declare_dram_parameter("papt", [NT * P128, PCOL], bf16, isOutput=False)
    gout = nc.declare_dram_parameter("gout", [P128, PCOL], f32, isOutput=True)

    with tile.TileContext(nc) as tc:
        with (
            tc.tile_pool(name="big", bufs=1) as big,
            tc.tile_pool(name="small", bufs=1) as small,
            tc.tile_pool(name="bndp", bufs=2) as bndp,
            tc.tile_pool(name="hbuf", bufs=3) as hbuf,
            tc.tile_pool(name="psz", bufs=2, space="PSUM") as pszp,
            tc.tile_pool(name="psG", bufs=1, space="PSUM") as psGp,
        ):
            # preload the activation-function table while DMAs run
            warm = small.tile([1, 2], f32)
            nc.vector.memset(warm[:], 0.0)
            nc.scalar.activation(out=warm[:], in_=warm[:],
                                 func=mybir.ActivationFunctionType.Copy)

            # ---------- loads (xt first: it gates the critical path) ----------
            TWH = (TW // 2 + 1) & ~1
            xt_t = big.tile([P128, TW], bf16, tag="xtb")
            nc.sync.dma_start(out=xt_t[:, 0:TWH], in_=xt[:, 0:TWH])
            nc.sync.dma_start(out=xt_t[:, TWH:TW], in_=xt[:, TWH:TW])
            gidx_t = small.tile([P128, JWT // 16], i16)
            nc.sync.dma_start(out=gidx_t[:], in_=gidx[:])
            bidx_t = small.tile([P128, NBT // 16], i16)
            nc.sync.dma_start(out=bidx_t[:], in_=bidx[:])
            selw_t = small.tile([P128, H], bf16)
            nc.sync.dma_start(out=selw_t[:], in_=selw[:])
            papt_t = big.tile([P128, NT * PCOL], bf16, tag="papt")
            for c in range(C):
                nc.sync.dma_start(
                    out=papt_t[:, (DOFF[c] // 128) * PCOL:
                               (DOFF[c + 1] // 128) * PCOL].rearrange(
                        "p (u j) -> p u j", j=PCOL
                    ),
                    in_=papt[DOFF[c] : DOFF[c + 1], :].rearrange(
                        "(u p) j -> p u j", p=P128
                    ),
                )

            # ---------- expand table to fp32 (split Act / DVE, per DMA half) ----
            table = big.tile([P128, TW], f32, tag="table")
            # balance: Act 0.833 ns/el vs DVE 0.521 ns/el -> Act share 0.385
            for h0, h1e in ((0, TWH), (TWH, TW)):
                XA = (h0 + int((h1e - h0) * 0.385)) & ~1
                nc.scalar.activation(
                    out=table[:, h0:XA], in_=xt_t[:, h0:XA],
                    func=mybir.ActivationFunctionType.Copy,
                )
                nc.vector.tensor_copy(out=table[:, XA:h1e], in_=xt_t[:, XA:h1e])

            # ---------- gather all streams ----------
            gath = big.tile([P128, JWT], f32, tag="gath")
            nc.gpsimd.ap_gather(
                out_ap=gath[:], in_ap=table[:], idxs_ap=gidx_t[:],
                channels=P128, num_elems=TW, d=1, num_idxs=JWT,
            )

            # ---------- per-chunk scan (in place) ----------
            for c in range(C):
                o0, o1 = int(OFF[c]), int(OFF[c + 1])
                nc.vector.tensor_tensor_scan(
                    out=gath[:, o0:o1], data0=gath[:, o0:o1], data1=gath[:, o0:o1],
                    initial=0.0, op0=mybir.AluOpType.add,
                    op1=mybir.AluOpType.bypass,
                )

            # ---------- per-chunk boundary gather ----------
            bnds = []
            for c in range(C):
                o0, o1 = int(OFF[c]), int(OFF[c + 1])
                bnd = bndp.tile([P128, NBCS[c]], f32, tag=f"bnd{c % 2}")
                nc.gpsimd.ap_gather(
                    out_ap=bnd[:], in_ap=gath[:, o0:o1],
                    idxs_ap=bidx_t[:, BOFF[c] // 16:BOFF[c + 1] // 16],
                    channels=P128, num_elems=int(JWS[c]), d=1, num_idxs=NBCS[c],
                )
                bnds.append(bnd)

            # ---------- per-chunk diff -> dt (bf16) ----------
            dt = big.tile([P128, NS], bf16, tag="dt")
            for c in range(C):
                d0, nd = DOFF[c], NDCS[c]
                nc.vector.tensor_tensor(
                    out=dt[:, d0:d0 + nd],
                    in0=bnds[c][:, 1:1 + nd], in1=bnds[c][:, 0:nd],
                    op=mybir.AluOpType.subtract,
                )

            # ---------- phase B: z -> relu -> G (sw-pipelined batches) ----------
            G_ps = psGp.tile([P128, PCOL], f32, tag="G")
            QB = 8
            batches = []
            for c in range(C):
                t = DOFF[c] // 128
                left = TCH[c]
                while left > 0:
                    sz = min(QB, left)
                    batches.append((t, sz))
                    t += sz
                    left -= sz

            def z_mms(t0, m, ps):
                for u in range(m):
                    n0 = (t0 + u) * P128
                    nc.tensor.matmul(
                        out=ps[:, u * H:(u + 1) * H],
                        lhsT=dt[:, n0:n0 + P128], rhs=selw_t[:],
                        start=True, stop=True,
                    )

            def g_mms(t0, m, h1):
                for u in range(m):
                    t = t0 + u
                    nc.tensor.matmul(
                        out=G_ps[:],
                        lhsT=h1[:, u * H:(u + 1) * H],
                        rhs=papt_t[:, t * PCOL:(t + 1) * PCOL],
                        start=(t == 0), stop=(t == NT - 1),
                    )

            prev = None
            for bi, (t0, m) in enumerate(batches):
                ps = pszp.tile([P128, QB * H], f32, tag="z")
                z_mms(t0, m, ps)
                h1 = hbuf.tile([P128, QB * H], bf16, tag="h1")
                nc.scalar.activation(
                    out=h1[:, :m * H], in_=ps[:, :m * H],
                    func=mybir.ActivationFunctionType.Relu,
                )
                if prev is not None:
                    g_mms(*prev)
                prev = (t0, m, h1)
            g_mms(*prev)

            G_sb = small.tile([P128, PCOL], f32)
            nc.vector.tensor_copy(out=G_sb[:], in_=G_ps[:])
            nc.sync.dma_start(out=gout[:], in_=G_sb[:])

    nc.compile()
    return nc


def _preprocess(x, edge_index, batch_idx):
    """Integer/structure preprocessing -> per-core device inputs."""
    src = np.asarray(edge_index[0], dtype=np.int64)
    dst = np.asarray(edge_index[1], dtype=np.int64)

    deg = (np.bincount(dst, minlength=N) + 1).astype(np.float32)
    dis = (1.0 / np.sqrt(deg)).astype(np.float32)
    sq = np.sqrt(deg).astype(np.float32)
    dis_pad = np.zeros(NPAD, np.float32)
    dis_pad[:N] = dis
    sq_pad = np.zeros(NPAD, np.float32)
    sq_pad[:N] = sq

    bi = np.asarray(batch_idx, dtype=np.int64)
    cnt = np.bincount(bi, minlength=B).astype(np.float32)

    x_np = np.asarray(x, dtype=np.float32)
    x_pad = np.zeros((NPAD, IN), np.float32)
    x_pad[:N] = x_np
    disx = x_pad * dis_pad[:, None]          # [NPAD, 6]

    # ---- pooling matrices (dense PA = P @ A) ----
    loop = np.arange(N, dtype=np.int64)
    src2 = np.concatenate([src, loop])
    dst2 = np.concatenate([dst, loop])
    w = (dis[src2] * dis[dst2]).astype(np.float64)
    flat = bi[dst2] * NPAD + src2
    PA = np.bincount(flat, weights=w, minlength=B * NPAD).reshape(B, NPAD)
    PA = PA.astype(np.float32)
    Pm = np.zeros((B, NPAD), np.float32)
    Pm[bi, np.arange(N)] = 1.0
    papt_full = (np.concatenate([PA, Pm], axis=0) * dis_pad[None, :]).T  # [NPAD,128]

    # graph span per core (for the P columns)
    first_graph = np.zeros(NG, np.int64)
    span = np.zeros(NG, np.int64)
    for k in range(NG):
        lo, hi = k * NS, min((k + 1) * NS, N)
        if lo >= N:
            first_graph[k] = B - 1
            span[k] = 1
            continue
        gset = bi[lo:hi]
        first_graph[k] = gset[0]
        span[k] = gset[-1] - gset[0] + 1
        assert span[k] <= PCOL - B, f"graph span {span[k]} > {PCOL - B}"

    # ---- per (core, bank) compacted streams (real edges + self entries) ----
    core = dst // NS
    bank = src // NS
    src_local = src - bank * NS
    dst_local = dst - core * NS
    # self pseudo-edges: dst d -> bank d_local % NG, payload col = ext slot
    d_all = np.arange(N, dtype=np.int64)
    core_sf = d_all // NS
    dstl_sf = d_all - core_sf * NS
    bank_sf = dstl_sf % NG
    core2 = np.concatenate([core, core_sf])
    bank2 = np.concatenate([bank, bank_sf])
    srcl2 = np.concatenate([src_local, dstl_sf // NG])   # ext pos for self
    dstl2 = np.concatenate([dst_local, dstl_sf])
    flag2 = np.concatenate([np.zeros(E, bool), np.ones(N, bool)])
    chunk2 = np.searchsorted(np.asarray(DOFF[1:]), dstl2, side="right")
    key = ((core2 * NG + bank2) * C + chunk2) * NS + dstl2
    order = np.argsort(key, kind="stable")
    srcl_s = srcl2[order]
    dstl_s = dstl2[order]
    flag_s = flag2[order]

    cell = ((core2 * NG + bank2) * C + chunk2)[order]
    cellcnt = np.bincount(cell, minlength=NG * NG * C)
    cell_starts = np.zeros(NG * NG * C + 1, np.int64)
    np.cumsum(cellcnt, out=cell_starts[1:])

    # compact column maps per (core, bank)
    colmaps = {}
    ncols = np.zeros((NG, NG), np.int64)
    for k in range(NG):
        for g in range(NG):
            s0 = cell_starts[(k * NG + g) * C]
            s1 = cell_starts[(k * NG + g + 1) * C]
            uniq = np.unique(srcl_s[s0:s1][~flag_s[s0:s1]])
            colmaps[(k, g)] = uniq
            ncols[k, g] = len(uniq)
    # ext region: NS/NG self columns per bank (self-loop feature + sqrt(deg)
    # rows; bank of dst d = d_local % NG, ext position = d_local // NG)
    EXT = NS // NG
    TW = int(ncols.max()) + 1 + EXT
    TW = (TW + 15) & ~15

    # per-chunk stream widths (shared across cores for one compiled NEFF)
    cc = cellcnt.reshape(NG, NG, C)
    JWS = []
    for c in range(C):
        m = int(cc[:, :, c].max())
        JWS.append(((m + 1 + 15) // 16) * 16)
    JWT = sum(JWS)
    OFF = np.concatenate([[0], np.cumsum(JWS)]).astype(int)

    # build tables / idx arrays per core
    xt_all = np.zeros((NG, P128, TW), ml_dtypes.bfloat16)
    gidx_all = np.zeros((NG, P128, JWT // 16), np.int16)
    bidx_all = np.zeros((NG, P128, NBT // 16), np.int16)

    for k in range(NG):
        for g in range(NG):
            uniq = colmaps[(k, g)]
            n0 = g * NS
            base = 1 + len(uniq)
            xt_all[k, 16 * g:16 * g + 6, 1:base] = (
                disx[n0 + uniq].T.astype(ml_dtypes.bfloat16)
            )
            # ext/self columns: ext slot j <-> dst d = k*NS + j*NG + g
            dsf = k * NS + np.arange(EXT, dtype=np.int64) * NG + g
            dsf = dsf[dsf < NPAD]
            xt_all[k, 16 * g:16 * g + 6, base:base + len(dsf)] = (
                disx[dsf].T.astype(ml_dtypes.bfloat16)
            )
            xt_all[k, 16 * g + 6, base:base + len(dsf)] = (
                sq_pad[dsf].astype(ml_dtypes.bfloat16)
            )
            # remap this (core, bank)'s stream to compact cols
            s0 = cell_starts[(k * NG + g) * C]
            s1 = cell_starts[(k * NG + g + 1) * C]
            sl = srcl_s[s0:s1]
            fl = flag_s[s0:s1]
            comp = np.where(fl, base + sl, np.searchsorted(uniq, sl) + 1)

            for c in range(C):
                c0 = cell_starts[(k * NG + g) * C + c]
                c1 = cell_starts[(k * NG + g) * C + c + 1]
                ncell = c1 - c0
                stream = np.zeros(JWS[c], np.int64)
                stream[1:1 + ncell] = comp[c0 - s0:c1 - s0]
                blk = stream.reshape(JWS[c] // 16, 16).T.astype(np.int16)
                gidx_all[k, 16 * g:16 * (g + 1), OFF[c] // 16:OFF[c + 1] // 16] = blk

                nd, nb = NDCS[c], NBCS[c]
                dloc = dstl_s[c0:c1] - DOFF[c]
                cnts = np.bincount(dloc, minlength=nd)
                blist = np.zeros(nb, np.int64)
                np.cumsum(cnts, out=blist[1:1 + nd])
                blist[1 + nd:] = blist[nd]
                bblk = blist.reshape(nb // 16, 16).T.astype(np.int16)
                bidx_all[k, 16 * g:16 * (g + 1),
                         BOFF[c] // 16:BOFF[c + 1] // 16] = bblk

    # papt per core: 64 PA cols + local P cols, blocked [NT*128, PCOL]
    papt_all = np.zeros((NG, NT * P128, PCOL), ml_dtypes.bfloat16)
    for k in range(NG):
        n0 = k * NS
        pk = np.zeros((NS, PCOL), np.float32)
        pk[:, :B] = papt_full[n0:n0 + NS, :B]
        b0, sp = first_graph[k], span[k]
        pk[:, B:B + sp] = papt_full[n0:n0 + NS, B + b0:B + b0 + sp]
        papt_all[k] = pk.astype(ml_dtypes.bfloat16)

    return {
        "JW": (TW, tuple(JWS)),
        "TW": TW,
        "JWS": JWS,
        "xt_all": xt_all,
        "gidx_all": gidx_all,
        "bidx_all": bidx_all,
        "papt_all": papt_all,
        "first_graph": first_graph,
        "span": span,
        "cnt": cnt,
    }


def _head(G, cnt, inputs):
    f = np.float32
    W2 = np.asarray(inputs["W2"], f)
    b2 = np.asarray(inputs["b2"], f)
    Wg = np.asarray(inputs["Wg"], f)
    bg = np.asarray(inputs["bg"], f)
    Et = np.asarray(inputs["Et"], f)
    Ek = np.asarray(inputs["Ek"], f)
    Ev = np.asarray(inputs["Ev"], f)
    Wp = np.asarray(inputs["Wp"], f)
    bp = np.asarray(inputs["bp"], f)
    Ekid = np.asarray(inputs["Ekid"], f)
    Wc = np.asarray(inputs["Wc"], f)
    bc = np.asarray(inputs["bc"], f)
    Wl = np.asarray(inputs["Wl"], f)
    bl = np.asarray(inputs["bl"], f)
    Wm1 = np.asarray(inputs["Wm1"], f)
    bm1 = np.asarray(inputs["bm1"], f)
    Wm2 = np.asarray(inputs["Wm2"], f)
    bm2 = np.asarray(inputs["bm2"], f)
    st = np.asarray(inputs["sol_type_idx"], np.int64)
    sk = np.asarray(inputs["sol_key_idx"], np.int64)
    sv = np.asarray(inputs["sol_val_idx"], np.int64)
    kid = np.asarray(inputs["kernel_id"], np.int64)
    cond = np.asarray(inputs["cond_vec"], f)
    loc = np.asarray(inputs["local_feats"], f)

    relu = lambda a: np.maximum(a, 0.0).astype(f)

    Ph2 = G[:B] @ W2 + cnt[:, None] * b2[None, :] + G[B:]
    g = (Ph2 / np.maximum(cnt, 1.0)[:, None]) @ Wg + bg

    seq_mean = np.concatenate(
        [Et[st].mean(axis=1), Ek[sk].mean(axis=1), Ev[sv].mean(axis=1)], axis=-1
    ).astype(f)
    p = relu(seq_mean @ Wp + bp)
    kvec = Ekid[kid]
    c = relu(cond @ Wc + bc)
    l = relu(loc @ Wl + bl)
    xf = np.concatenate([g, p, kvec, c, l], axis=1).astype(f)
    return (relu(xf @ Wm1 + bm1) @ Wm2 + bm2).astype(f)


def kernel(**inputs) -> np.ndarray:
    from concourse.bass_utils import run_bass_kernel_spmd

    pre = _preprocess(inputs["x"], inputs["edge_index"], inputs["batch_idx"])
    sig = pre["JW"]
    if sig not in _compiled:
        _compiled[sig] = _build_nc(pre["TW"], tuple(pre["JWS"]))
    nc = _compiled[sig]

    W1 = np.asarray(inputs["W1"], np.float32)
    b1 = np.asarray(inputs["b1"], np.float32)
    selw = np.zeros((P128, H), ml_dtypes.bfloat16)
    for g in range(NG):
        selw[16 * g:16 * g + 6] = W1.astype(ml_dtypes.bfloat16)
        selw[16 * g + 6] = b1.astype(ml_dtypes.bfloat16)

    in_maps = []
    for k in range(NG):
        in_maps.append({
            "xt": pre["xt_all"][k],
            "gidx": pre["gidx_all"][k],
            "bidx": pre["bidx_all"][k],
            "selw": selw,
            "papt": pre["papt_all"][k],
        })

    res = run_bass_kernel_spmd(nc, in_maps, core_ids=list(range(NG)))

    Gpa = np.zeros((B, H), np.float64)
    Gp = np.zeros((B, H), np.float64)
    for k, r in enumerate(res.results):
        gt = r["gout"].astype(np.float64)      # [128 f, 80 c]
        Gpa += gt[:, :B].T
        b0, sp = pre["first_graph"][k], pre["span"][k]
        Gp[b0:b0 + sp] += gt[:, B:B + sp].T
    G = np.concatenate([Gpa, Gp], axis=0).astype(np.float32)   # [128, H]

    return _head(G, pre["cnt"], inputs)


# revision 25
# speedup vs baseline: 1.0561x; 1.0561x over previous
"""Trainium2 Bass kernel for nn_CryptoGNN (2-layer GCN + pooled heads).

Math (same collapse as validated baseline):
  With A = normalized adjacency (incl. self loops), P = [B,N] pooling,
  u[d] = sum_{s->d} dis[s]x[s] + dis[d]x[d],  zhat = u@W1 + sq*b1,
  h1 = dis*relu(zhat) = dis*relu(zhat);  relu commutes with dis>0 so
  G = [PA;P]^T-pooled h1 uses papt columns pre-scaled by dis and
  h1hat = relu(zhat) on device.  Layer 2 + heads collapse to the host
  (tiny [64,*] math).

Per-core device pipeline (8-way node sharding, 12544 dst nodes/core):
  1. bf16 compacted src table DMA ([128, TW]: 8 banks x 6 feature rows,
     only srcs with >=1 edge into this core; all 128 rows host-written
     so no SBUF garbage can reach the PE - 0*NaN != 0 on TRN2 PE)
  2. expand bf16->fp32 split across Act/DVE (gather needs 4B elems)
  3. one GPSIMD ap_gather of all dst-sorted per-bank edge streams
  4. per dst-chunk (7 chunks x 1792 dsts): fp32 prefix scan (DVE),
     boundary ap_gather (GPSIMD), shifted diff -> dt bf16 (DVE 2x mode)
  5. per node tile: z = dt_t^T @ selW + aug_t^T @ w1aug   (bf16 PE,
     selW = bank-scattered W1 rows; fold+mm1 fused, no PSUM->SBUF hop)
  6. relu -> h1 bf16 (Act), G^T += h1_t^T @ papt_t accumulated in one
     [128, 80] PSUM across all 98 tiles; single gout DMA.
Host sums the 8 partial G^T and runs the small head in numpy.
"""

import sys

if "/opt/trn_rl_repo" not in sys.path:
    sys.path.insert(0, "/opt/trn_rl_repo")

import numpy as np
import ml_dtypes

N = 100000
E = 600000
B = 64
IN = 6
H = 128
S = 16

NG = 8                    # banks (= src chunks) and cores
NS = 12544                # nodes per core shard (98*128)
NPAD = NS * NG            # 100352
NT = 98                   # node tiles per shard
# dst chunks per core, in node tiles; last chunk small to shrink the tail
TCH = (15, 15, 15, 15, 15, 15, 8)
C = len(TCH)
NDCS = tuple(t * 128 for t in TCH)            # dsts per chunk
# boundary widths: +32 keeps int16 idx slice offsets 4-byte aligned
NBCS = tuple(n + 32 for n in NDCS)
DOFF = tuple(int(x) for x in np.concatenate([[0], np.cumsum(NDCS)]))
BOFF = tuple(int(x) for x in np.concatenate([[0], np.cumsum(NBCS)]))
NBT = BOFF[-1]
PCOL = 80                 # papt columns: 64 PA + <=16 local P
P128 = 128

_compiled = {}


def _build_nc(TW, JWS):
    import concourse.bacc as bacc
    import concourse.mybir as mybir
    from concourse import tile

    f32 = mybir.dt.float32
    bf16 = mybir.dt.bfloat16
    i16 = mybir.dt.int16

    JWT = sum(JWS)
    OFF = np.concatenate([[0], np.cumsum(JWS)]).astype(int)

    nc = bacc.Bacc("TRN2", target_bir_lowering=False, debug=False)

    xt = nc.declare_dram_parameter("xt", [P128, TW], bf16, isOutput=False)
    gidx = nc.declare_dram_parameter("gidx", [P128, JWT // 16], i16, isOutput=False)
    bidx = nc.declare_dram_parameter("bidx", [P128, NBT // 16], i16, isOutput=False)
    selw = nc.declare_dram_parameter("selw", [P128, H], bf16, isOutput=False)
    papt = nc.declare_dram_parameter("papt", [NT * P128, PCOL], bf16, isOutput=False)
    gout = nc.declare_dram_parameter("gout", [P128, PCOL], f32, isOutput=True)

    with tile.TileContext(nc) as tc:
        with (
            tc.tile_pool(name="big", bufs=1) as big,
            tc.tile_pool(name="small", bufs=1) as small,
            tc.tile_pool(name="bndp", bufs=2) as bndp,
            tc.tile_pool(name="hbuf", bufs=3) as hbuf,
            tc.tile_pool(name="psz", bufs=2, space="PSUM") as pszp,
            tc.tile_pool(name="psG", bufs=1, space="PSUM") as psGp,
        ):
            # preload the activation-function table while DMAs run
            warm = small.tile([1, 2], f32)
            nc.vector.memset(warm[:], 0.0)
            nc.scalar.activation(out=warm[:], in_=warm[:],
                                 func=mybir.ActivationFunctionType.Copy)

            # ---------- loads (xt first: it gates the critical path) ----------
            TWH = (TW // 2 + 1) & ~1
            xt_t = big.tile([P128, TW], bf16, tag="xtb")
            nc.sync.dma_start(out=xt_t[:, 0:TWH], in_=xt[:, 0:TWH])
            nc.sync.dma_start(out=xt_t[:, TWH:TW], in_=xt[:, TWH:TW])
            gidx_t = small.tile([P128, JWT // 16], i16)
            nc.sync.dma_start(out=gidx_t[:], in_=gidx[:])
            bidx_t = small.tile([P128, NBT // 16], i16)
            nc.sync.dma_start(out=bidx_t[:], in_=bidx[:])
            selw_t = small.tile([P128, H], bf16)
            nc.sync.dma_start(out=selw_t[:], in_=selw[:])
            papt_t = big.tile([P128, NT * PCOL], bf16, tag="papt")
            for c in range(C):
                nc.sync.dma_start(
                    out=papt_t[:, (DOFF[c] // 128) * PCOL:
                               (DOFF[c + 1] // 128) * PCOL].rearrange(
                        "p (u j) -> p u j", j=PCOL
                    ),
                    in_=papt[DOFF[c] : DOFF[c + 1], :].rearrange(
                        "(u p) j -> p u j", p=P128
                    ),
                )

            # ---------- expand table to fp32 (split Act / DVE, per DMA half) ----
            table = big.tile([P128, TW], f32, tag="table")
            # balance: Act 0.833 ns/el vs DVE 0.521 ns/el -> Act share 0.385
            for h0, h1e in ((0, TWH), (TWH, TW)):
                XA = (h0 + int((h1e - h0) * 0.385)) & ~1
                nc.scalar.activation(
                    out=table[:, h0:XA], in_=xt_t[:, h0:XA],
                    func=mybir.ActivationFunctionType.Copy,
                )
                nc.vector.tensor_copy(out=table[:, XA:h1e], in_=xt_t[:, XA:h1e])

            # ---------- gather all streams ----------
            gath = big.tile([P128, JWT], f32, tag="gath")
            nc.gpsimd.ap_gather(
                out_ap=gath[:], in_ap=table[:], idxs_ap=gidx_t[:],
                channels=P128, num_elems=TW, d=1, num_idxs=JWT,
            )

            # ---------- per-chunk scan / boundary gather / diff ----------
            # DVE program order interleaves scans and diffs (scans lead by 2)
            # so diff_c runs as soon as g2_c lands instead of after all scans.
            dt = big.tile([P128, NS], bf16, tag="dt")
            bnds = [None] * C

            def scan_c(c):
                o0, o1 = int(OFF[c]), int(OFF[c + 1])
                nc.vector.tensor_tensor_scan(
                    out=gath[:, o0:o1], data0=gath[:, o0:o1], data1=gath[:, o0:o1],
                    initial=0.0, op0=mybir.AluOpType.add,
                    op1=mybir.AluOpType.bypass,
                )

            def g2_c(c):
                o0, o1 = int(OFF[c]), int(OFF[c + 1])
                bnd = bndp.tile([P128, NBCS[c]], f32, tag=f"bnd{c % 2}",
                                name=f"bnd_{c}")
                nc.gpsimd.ap_gather(
                    out_ap=bnd[:], in_ap=gath[:, o0:o1],
                    idxs_ap=bidx_t[:, BOFF[c] // 16:BOFF[c + 1] // 16],
                    channels=P128, num_elems=int(JWS[c]), d=1, num_idxs=NBCS[c],
                )
                bnds[c] = bnd

            def diff_c(c):
                d0, nd = DOFF[c], NDCS[c]
                nc.vector.tensor_tensor(
                    out=dt[:, d0:d0 + nd],
                    in0=bnds[c][:, 1:1 + nd], in1=bnds[c][:, 0:nd],
                    op=mybir.AluOpType.subtract,
                )

            scan_c(0)
            scan_c(1)
            g2_c(0)
            g2_c(1)
            for c in range(2, C):
                scan_c(c)
                g2_c(c)
                diff_c(c - 2)
            diff_c(C - 2)
            diff_c(C - 1)

            # ---------- phase B: z -> relu -> G (sw-pipelined batches) ----------
            G_ps = psGp.tile([P128, PCOL], f32, tag="G")
            QB = 8
            batches = []
            for c in range(C):
                t = DOFF[c] // 128
                left = TCH[c]
                while left > 0:
                    sz = min(QB, left)
                    batches.append((t, sz))
                    t += sz
                    left -= sz

            def z_mms(t0, m, ps):
                for u in range(m):
                    n0 = (t0 + u) * P128
                    nc.tensor.matmul(
                        out=ps[:, u * H:(u + 1) * H],
                        lhsT=dt[:, n0:n0 + P128], rhs=selw_t[:],
                        start=True, stop=True,
                    )

            def g_mms(t0, m, h1):
                for u in range(m):
                    t = t0 + u
                    nc.tensor.matmul(
                        out=G_ps[:],
                        lhsT=h1[:, u * H:(u + 1) * H],
                        rhs=papt_t[:, t * PCOL:(t + 1) * PCOL],
                        start=(t == 0), stop=(t == NT - 1),
                    )

            prev = None
            for bi, (t0, m) in enumerate(batches):
                ps = pszp.tile([P128, QB * H], f32, tag="z")
                z_mms(t0, m, ps)
                h1 = hbuf.tile([P128, QB * H], bf16, tag="h1")
                nc.scalar.activation(
                    out=h1[:, :m * H], in_=ps[:, :m * H],
                    func=mybir.ActivationFunctionType.Relu,
                )
                if prev is not None:
                    g_mms(*prev)
                prev = (t0, m, h1)
            g_mms(*prev)

            G_sb = small.tile([P128, PCOL], f32)
            nc.vector.tensor_copy(out=G_sb[:], in_=G_ps[:])
            nc.sync.dma_start(out=gout[:], in_=G_sb[:])

    nc.compile()
    return nc


def _preprocess(x, edge_index, batch_idx):
    """Integer/structure preprocessing -> per-core device inputs."""
    src = np.asarray(edge_index[0], dtype=np.int64)
    dst = np.asarray(edge_index[1], dtype=np.int64)

    deg = (np.bincount(dst, minlength=N) + 1).astype(np.float32)
    dis = (1.0 / np.sqrt(deg)).astype(np.float32)
    sq = np.sqrt(deg).astype(np.float32)
    dis_pad = np.zeros(NPAD, np.float32)
    dis_pad[:N] = dis
    sq_pad = np.zeros(NPAD, np.float32)
    sq_pad[:N] = sq

    bi = np.asarray(batch_idx, dtype=np.int64)
    cnt = np.bincount(bi, minlength=B).astype(np.float32)

    x_np = np.asarray(x, dtype=np.float32)
    x_pad = np.zeros((NPAD, IN), np.float32)
    x_pad[:N] = x_np
    disx = x_pad * dis_pad[:, None]          # [NPAD, 6]

    # ---- pooling matrices (dense PA = P @ A) ----
    loop = np.arange(N, dtype=np.int64)
    src2 = np.concatenate([src, loop])
    dst2 = np.concatenate([dst, loop])
    w = (dis[src2] * dis[dst2]).astype(np.float64)
    flat = bi[dst2] * NPAD + src2
    PA = np.bincount(flat, weights=w, minlength=B * NPAD).reshape(B, NPAD)
    PA = PA.astype(np.float32)
    Pm = np.zeros((B, NPAD), np.float32)
    Pm[bi, np.arange(N)] = 1.0
    papt_full = (np.concatenate([PA, Pm], axis=0) * dis_pad[None, :]).T  # [NPAD,128]

    # graph span per core (for the P columns)
    first_graph = np.zeros(NG, np.int64)
    span = np.zeros(NG, np.int64)
    for k in range(NG):
        lo, hi = k * NS, min((k + 1) * NS, N)
        if lo >= N:
            first_graph[k] = B - 1
            span[k] = 1
            continue
        gset = bi[lo:hi]
        first_graph[k] = gset[0]
        span[k] = gset[-1] - gset[0] + 1
        assert span[k] <= PCOL - B, f"graph span {span[k]} > {PCOL - B}"

    # ---- per (core, bank) compacted streams (real edges + self entries) ----
    core = dst // NS
    bank = src // NS
    src_local = src - bank * NS
    dst_local = dst - core * NS
    # self pseudo-edges: one per real dst node.  Assign each to a bank by
    # waterfilling the (core, bank, chunk) cell counts so the padded per-bank
    # stream width (max cell) stays near the mean.
    d_all = np.arange(N, dtype=np.int64)
    core_sf = d_all // NS
    dstl_sf = d_all - core_sf * NS
    chunk_r = np.searchsorted(np.asarray(DOFF[1:]), dst_local, side="right")
    rc = np.bincount((core * NG + bank) * C + chunk_r,
                     minlength=NG * NG * C).reshape(NG, NG, C)
    chunk_sf = np.searchsorted(np.asarray(DOFF[1:]), dstl_sf, side="right")
    bank_sf = np.empty(N, np.int64)
    for k in range(NG):
        for c in range(C):
            sel = (core_sf == k) & (chunk_sf == c)
            m = int(sel.sum())
            if m == 0:
                continue
            cnts = rc[k, :, c].astype(np.float64)
            # waterfill: find level T with sum(max(0,T-cnts))=m
            lo, hi = cnts.min(), cnts.max() + m
            for _ in range(60):
                mid = 0.5 * (lo + hi)
                if np.maximum(0.0, mid - cnts).sum() >= m:
                    hi = mid
                else:
                    lo = mid
            fill = np.maximum(0.0, hi - cnts).astype(np.int64)
            # fix rounding to sum exactly m
            while fill.sum() > m:
                fill[np.argmax(cnts + fill)] -= 1
            while fill.sum() < m:
                fill[np.argmin(cnts + fill)] += 1
            bank_sf[sel] = np.repeat(np.arange(NG), fill)
    # ext position: rank of each self dst within its (core, bank), dst order
    gkey = core_sf * NG + bank_sf
    ordg = np.argsort(gkey * np.int64(NPAD) + d_all, kind="stable")
    gsorted = gkey[ordg]
    starts = np.searchsorted(gsorted, np.arange(NG * NG))
    extpos = np.empty(N, np.int64)
    extpos[ordg] = np.arange(N) - starts[gsorted]
    selfcnt = np.bincount(gkey, minlength=NG * NG).reshape(NG, NG)

    core2 = np.concatenate([core, core_sf])
    bank2 = np.concatenate([bank, bank_sf])
    srcl2 = np.concatenate([src_local, extpos])          # ext pos for self
    dstl2 = np.concatenate([dst_local, dstl_sf])
    flag2 = np.concatenate([np.zeros(E, bool), np.ones(N, bool)])
    chunk2 = np.searchsorted(np.asarray(DOFF[1:]), dstl2, side="right")
    key = ((core2 * NG + bank2) * C + chunk2) * NS + dstl2
    order = np.argsort(key, kind="stable")
    srcl_s = srcl2[order]
    dstl_s = dstl2[order]
    flag_s = flag2[order]

    cell = ((core2 * NG + bank2) * C + chunk2)[order]
    cellcnt = np.bincount(cell, minlength=NG * NG * C)
    cell_starts = np.zeros(NG * NG * C + 1, np.int64)
    np.cumsum(cellcnt, out=cell_starts[1:])

    # compact column maps per (core, bank)
    colmaps = {}
    ncols = np.zeros((NG, NG), np.int64)
    for k in range(NG):
        for g in range(NG):
            s0 = cell_starts[(k * NG + g) * C]
            s1 = cell_starts[(k * NG + g + 1) * C]
            uniq = np.unique(srcl_s[s0:s1][~flag_s[s0:s1]])
            colmaps[(k, g)] = uniq
            ncols[k, g] = len(uniq)
    # ext region: per-(core,bank) self columns (self-loop features + sqrt(deg))
    TW = int((ncols + selfcnt).max()) + 1
    TW = (TW + 15) & ~15

    # per-chunk stream widths (shared across cores for one compiled NEFF)
    cc = cellcnt.reshape(NG, NG, C)
    JWS = []
    for c in range(C):
        m = int(cc[:, :, c].max())
        JWS.append(((m + 1 + 15) // 16) * 16)
    JWT = sum(JWS)
    OFF = np.concatenate([[0], np.cumsum(JWS)]).astype(int)

    # build tables / idx arrays per core
    xt_all = np.zeros((NG, P128, TW), ml_dtypes.bfloat16)
    gidx_all = np.zeros((NG, P128, JWT // 16), np.int16)
    bidx_all = np.zeros((NG, P128, NBT // 16), np.int16)

    for k in range(NG):
        for g in range(NG):
            uniq = colmaps[(k, g)]
            n0 = g * NS
            base = 1 + len(uniq)
            xt_all[k, 16 * g:16 * g + 6, 1:base] = (
                disx[n0 + uniq].T.astype(ml_dtypes.bfloat16)
            )
            # ext/self columns: this (core,bank)'s self dsts in dst order
            gi = k * NG + g
            dsf = d_all[ordg[starts[gi]:starts[gi] + selfcnt[k, g]]]
            xt_all[k, 16 * g:16 * g + 6, base:base + len(dsf)] = (
                disx[dsf].T.astype(ml_dtypes.bfloat16)
            )
            xt_all[k, 16 * g + 6, base:base + len(dsf)] = (
                sq_pad[dsf].astype(ml_dtypes.bfloat16)
            )
            # remap this (core, bank)'s stream to compact cols
            s0 = cell_starts[(k * NG + g) * C]
            s1 = cell_starts[(k * NG + g + 1) * C]
            sl = srcl_s[s0:s1]
            fl = flag_s[s0:s1]
            comp = np.where(fl, base + sl, np.searchsorted(uniq, sl) + 1)

            for c in range(C):
                c0 = cell_starts[(k * NG + g) * C + c]
                c1 = cell_starts[(k * NG + g) * C + c + 1]
                ncell = c1 - c0
                stream = np.zeros(JWS[c], np.int64)
                stream[1:1 + ncell] = comp[c0 - s0:c1 - s0]
                blk = stream.reshape(JWS[c] // 16, 16).T.astype(np.int16)
                gidx_all[k, 16 * g:16 * (g + 1), OFF[c] // 16:OFF[c + 1] // 16] = blk

                nd, nb = NDCS[c], NBCS[c]
                dloc = dstl_s[c0:c1] - DOFF[c]
                cnts = np.bincount(dloc, minlength=nd)
                blist = np.zeros(nb, np.int64)
                np.cumsum(cnts, out=blist[1:1 + nd])
                blist[1 + nd:] = blist[nd]
                bblk = blist.reshape(nb // 16, 16).T.astype(np.int16)
                bidx_all[k, 16 * g:16 * (g + 1),
                         BOFF[c] // 16:BOFF[c + 1] // 16] = bblk

    # papt per core: 64 PA cols + local P cols, blocked [NT*128, PCOL]
    papt_all = np.zeros((NG, NT * P128, PCOL), ml_dtypes.bfloat16)
    for k in range(NG):
        n0 = k * NS
        pk = np.zeros((NS, PCOL), np.float32)
        pk[:, :B] = papt_full[n0:n0 + NS, :B]
        b0, sp = first_graph[k], span[k]
        pk[:, B:B + sp] = papt_full[n0:n0 + NS, B + b0:B + b0 + sp]
        papt_all[k] = pk.astype(ml_dtypes.bfloat16)

    return {
        "JW": (TW, tuple(JWS)),
        "TW": TW,
        "JWS": JWS,
        "xt_all": xt_all,
        "gidx_all": gidx_all,
        "bidx_all": bidx_all,
        "papt_all": papt_all,
        "first_graph": first_graph,
        "span": span,
        "cnt": cnt,
    }


def _head(G, cnt, inputs):
    f = np.float32
    W2 = np.asarray(inputs["W2"], f)
    b2 = np.asarray(inputs["b2"], f)
    Wg = np.asarray(inputs["Wg"], f)
    bg = np.asarray(inputs["bg"], f)
    Et = np.asarray(inputs["Et"], f)
    Ek = np.asarray(inputs["Ek"], f)
    Ev = np.asarray(inputs["Ev"], f)
    Wp = np.asarray(inputs["Wp"], f)
    bp = np.asarray(inputs["bp"], f)
    Ekid = np.asarray(inputs["Ekid"], f)
    Wc = np.asarray(inputs["Wc"], f)
    bc = np.asarray(inputs["bc"], f)
    Wl = np.asarray(inputs["Wl"], f)
    bl = np.asarray(inputs["bl"], f)
    Wm1 = np.asarray(inputs["Wm1"], f)
    bm1 = np.asarray(inputs["bm1"], f)
    Wm2 = np.asarray(inputs["Wm2"], f)
    bm2 = np.asarray(inputs["bm2"], f)
    st = np.asarray(inputs["sol_type_idx"], np.int64)
    sk = np.asarray(inputs["sol_key_idx"], np.int64)
    sv = np.asarray(inputs["sol_val_idx"], np.int64)
    kid = np.asarray(inputs["kernel_id"], np.int64)
    cond = np.asarray(inputs["cond_vec"], f)
    loc = np.asarray(inputs["local_feats"], f)

    relu = lambda a: np.maximum(a, 0.0).astype(f)

    Ph2 = G[:B] @ W2 + cnt[:, None] * b2[None, :] + G[B:]
    g = (Ph2 / np.maximum(cnt, 1.0)[:, None]) @ Wg + bg

    seq_mean = np.concatenate(
        [Et[st].mean(axis=1), Ek[sk].mean(axis=1), Ev[sv].mean(axis=1)], axis=-1
    ).astype(f)
    p = relu(seq_mean @ Wp + bp)
    kvec = Ekid[kid]
    c = relu(cond @ Wc + bc)
    l = relu(loc @ Wl + bl)
    xf = np.concatenate([g, p, kvec, c, l], axis=1).astype(f)
    return (relu(xf @ Wm1 + bm1) @ Wm2 + bm2).astype(f)


def kernel(**inputs) -> np.ndarray:
    from concourse.bass_utils import run_bass_kernel_spmd

    pre = _preprocess(inputs["x"], inputs["edge_index"], inputs["batch_idx"])
    sig = pre["JW"]
    if sig not in _compiled:
        _compiled[sig] = _build_nc(pre["TW"], tuple(pre["JWS"]))
    nc = _compiled[sig]

    W1 = np.asarray(inputs["W1"], np.float32)
    b1 = np.asarray(inputs["b1"], np.float32)
    selw = np.zeros((P128, H), ml_dtypes.bfloat16)
    for g in range(NG):
        selw[16 * g:16 * g + 6] = W1.astype(ml_dtypes.bfloat16)
        selw[16 * g + 6] = b1.astype(ml_dtypes.bfloat16)

    in_maps = []
    for k in range(NG):
        in_maps.append({
            "xt": pre["xt_all"][k],
            "gidx": pre["gidx_all"][k],
            "bidx": pre["bidx_all"][k],
            "selw": selw,
            "papt": pre["papt_all"][k],
        })

    res = run_bass_kernel_spmd(nc, in_maps, core_ids=list(range(NG)))

    Gpa = np.zeros((B, H), np.float64)
    Gp = np.zeros((B, H), np.float64)
    for k, r in enumerate(res.results):
        gt = r["gout"].astype(np.float64)      # [128 f, 80 c]
        Gpa += gt[:, :B].T
        b0, sp = pre["first_graph"][k], pre["span"][k]
        Gp[b0:b0 + sp] += gt[:, B:B + sp].T
    G = np.concatenate([Gpa, Gp], axis=0).astype(np.float32)   # [128, H]

    return _head(G, pre["cnt"], inputs)


# revision 36
# speedup vs baseline: 1.1037x; 1.0451x over previous
"""Trainium2 Bass kernel for nn_CryptoGNN (2-layer GCN + pooled heads).

Math (same collapse as validated baseline):
  With A = normalized adjacency (incl. self loops), P = [B,N] pooling,
  u[d] = sum_{s->d} dis[s]x[s] + dis[d]x[d],  zhat = u@W1 + sq*b1,
  h1 = dis*relu(zhat) = dis*relu(zhat);  relu commutes with dis>0 so
  G = [PA;P]^T-pooled h1 uses papt columns pre-scaled by dis and
  h1hat = relu(zhat) on device.  Layer 2 + heads collapse to the host
  (tiny [64,*] math).

Per-core device pipeline (8-way node sharding, 12544 dst nodes/core):
  1. bf16 compacted src table DMA ([128, TW]: 8 banks x 6 feature rows,
     only srcs with >=1 edge into this core; all 128 rows host-written
     so no SBUF garbage can reach the PE - 0*NaN != 0 on TRN2 PE)
  2. expand bf16->fp32 split across Act/DVE (gather needs 4B elems)
  3. one GPSIMD ap_gather of all dst-sorted per-bank edge streams
  4. per dst-chunk (7 chunks x 1792 dsts): fp32 prefix scan (DVE),
     boundary ap_gather (GPSIMD), shifted diff -> dt bf16 (DVE 2x mode)
  5. per node tile: z = dt_t^T @ selW + aug_t^T @ w1aug   (bf16 PE,
     selW = bank-scattered W1 rows; fold+mm1 fused, no PSUM->SBUF hop)
  6. relu -> h1 bf16 (Act), G^T += h1_t^T @ papt_t accumulated in one
     [128, 80] PSUM across all 98 tiles; single gout DMA.
Host sums the 8 partial G^T and runs the small head in numpy.
"""

import sys

if "/opt/trn_rl_repo" not in sys.path:
    sys.path.insert(0, "/opt/trn_rl_repo")

import numpy as np
import ml_dtypes

N = 100000
E = 600000
B = 64
IN = 6
H = 128
S = 16

NG = 8                    # banks (= src chunks) and cores
NS = 12544                # nodes per core shard (98*128)
NPAD = NS * NG            # 100352
NT = 98                   # node tiles per shard
# dst chunks per core, in node tiles; last chunk small to shrink the tail
TCH = (16, 16, 16, 16, 16, 14, 4)
C = len(TCH)
NDCS = tuple(t * 128 for t in TCH)            # dsts per chunk
# boundary widths: +32 keeps int16 idx slice offsets 4-byte aligned
NBCS = tuple(n + 32 for n in NDCS)
DOFF = tuple(int(x) for x in np.concatenate([[0], np.cumsum(NDCS)]))
BOFF = tuple(int(x) for x in np.concatenate([[0], np.cumsum(NBCS)]))
NBT = BOFF[-1]
PCOL = 80                 # papt columns: 64 PA + <=16 local P
P128 = 128

_compiled = {}


def _build_nc(TW, JWS):
    import concourse.bacc as bacc
    import concourse.mybir as mybir
    from concourse import tile

    f32 = mybir.dt.float32
    bf16 = mybir.dt.bfloat16
    i16 = mybir.dt.int16

    JWT = sum(JWS)
    OFF = np.concatenate([[0], np.cumsum(JWS)]).astype(int)

    nc = bacc.Bacc("TRN2", target_bir_lowering=False, debug=False)

    xt = nc.declare_dram_parameter("xt", [P128, TW], bf16, isOutput=False)
    gidx = nc.declare_dram_parameter("gidx", [P128, JWT // 16], i16, isOutput=False)
    bidx = nc.declare_dram_parameter("bidx", [P128, NBT // 16], i16, isOutput=False)
    aug = nc.declare_dram_parameter("aug", [7, NS], bf16, isOutput=False)
    selw = nc.declare_dram_parameter("selw", [P128, H], bf16, isOutput=False)
    w1aug = nc.declare_dram_parameter("w1aug", [7, H], bf16, isOutput=False)
    papt = nc.declare_dram_parameter("papt", [NT * P128, PCOL], bf16, isOutput=False)
    gout = nc.declare_dram_parameter("gout", [P128, PCOL], f32, isOutput=True)

    with tile.TileContext(nc) as tc:
        with (
            tc.tile_pool(name="big", bufs=1) as big,
            tc.tile_pool(name="small", bufs=1) as small,
            tc.tile_pool(name="bndp", bufs=2) as bndp,
            tc.tile_pool(name="hbuf", bufs=3) as hbuf,
            tc.tile_pool(name="psz", bufs=2, space="PSUM") as pszp,
            tc.tile_pool(name="psG", bufs=1, space="PSUM") as psGp,
        ):
            # preload the activation-function table while DMAs run
            warm = small.tile([1, 2], f32)
            nc.vector.memset(warm[:], 0.0)
            nc.scalar.activation(out=warm[:], in_=warm[:],
                                 func=mybir.ActivationFunctionType.Copy)

            # ---------- loads (xt first: it gates the critical path) ----------
            # xt in quarters, each expanded to fp32 as it lands (Act || DVE)
            NQ = 4
            qb = [(i * TW // NQ) & ~1 for i in range(NQ)] + [TW]
            xt_t = big.tile([P128, TW], bf16, tag="xtb")
            table = big.tile([P128, TW], f32, tag="table")
            for i in range(NQ):
                nc.sync.dma_start(out=xt_t[:, qb[i]:qb[i + 1]],
                                  in_=xt[:, qb[i]:qb[i + 1]])
                XA = (qb[i] + int((qb[i + 1] - qb[i]) * 0.385)) & ~1
                nc.scalar.activation(
                    out=table[:, qb[i]:XA], in_=xt_t[:, qb[i]:XA],
                    func=mybir.ActivationFunctionType.Copy,
                )
                nc.vector.tensor_copy(out=table[:, XA:qb[i + 1]],
                                      in_=xt_t[:, XA:qb[i + 1]])
            gidx_t = small.tile([P128, JWT // 16], i16)
            nc.sync.dma_start(out=gidx_t[:], in_=gidx[:])
            bidx_t = small.tile([P128, NBT // 16], i16)
            nc.sync.dma_start(out=bidx_t[:], in_=bidx[:])
            aug_t = small.tile([7, NS], bf16)
            nc.sync.dma_start(out=aug_t[:], in_=aug[:])
            selw_t = small.tile([P128, H], bf16)
            nc.sync.dma_start(out=selw_t[:], in_=selw[:])
            w1_t = small.tile([7, H], bf16)
            nc.sync.dma_start(out=w1_t[:], in_=w1aug[:])
            papt_t = big.tile([P128, NT * PCOL], bf16, tag="papt")
            for c in range(C):
                nc.sync.dma_start(
                    out=papt_t[:, (DOFF[c] // 128) * PCOL:
                               (DOFF[c + 1] // 128) * PCOL].rearrange(
                        "p (u j) -> p u j", j=PCOL
                    ),
                    in_=papt[DOFF[c] : DOFF[c + 1], :].rearrange(
                        "(u p) j -> p u j", p=P128
                    ),
                )

            # ---------- gather all streams ----------
            gath = big.tile([P128, JWT], f32, tag="gath")
            nc.gpsimd.ap_gather(
                out_ap=gath[:], in_ap=table[:], idxs_ap=gidx_t[:],
                channels=P128, num_elems=TW, d=1, num_idxs=JWT,
            )

            # ---------- per-chunk scan / boundary gather / diff ----------
            # DVE program order interleaves scans and diffs (scans lead by 2)
            # so diff_c runs as soon as g2_c lands instead of after all scans.
            dt = big.tile([P128, NS], bf16, tag="dt")
            bnds = [None] * C

            def scan_c(c):
                o0, o1 = int(OFF[c]), int(OFF[c + 1])
                nc.vector.tensor_tensor_scan(
                    out=gath[:, o0:o1], data0=gath[:, o0:o1], data1=gath[:, o0:o1],
                    initial=0.0, op0=mybir.AluOpType.add,
                    op1=mybir.AluOpType.bypass,
                )

            def g2_c(c):
                o0, o1 = int(OFF[c]), int(OFF[c + 1])
                bnd = bndp.tile([P128, NBCS[c]], f32, tag=f"bnd{c % 2}",
                                name=f"bnd_{c}")
                nc.gpsimd.ap_gather(
                    out_ap=bnd[:], in_ap=gath[:, o0:o1],
                    idxs_ap=bidx_t[:, BOFF[c] // 16:BOFF[c + 1] // 16],
                    channels=P128, num_elems=int(JWS[c]), d=1, num_idxs=NBCS[c],
                )
                bnds[c] = bnd

            def diff_c(c):
                d0, nd = DOFF[c], NDCS[c]
                nc.vector.tensor_tensor(
                    out=dt[:, d0:d0 + nd],
                    in0=bnds[c][:, 1:1 + nd], in1=bnds[c][:, 0:nd],
                    op=mybir.AluOpType.subtract,
                )

            scan_c(0)
            scan_c(1)
            g2_c(0)
            g2_c(1)
            for c in range(2, C):
                scan_c(c)
                g2_c(c)
                diff_c(c - 2)
            diff_c(C - 2)
            diff_c(C - 1)

            # ---------- phase B: z -> relu -> G (sw-pipelined batches) ----------
            G_ps = psGp.tile([P128, PCOL], f32, tag="G")
            QB = 8
            batches = []
            for c in range(C):
                t = DOFF[c] // 128
                left = TCH[c]
                while left > 0:
                    sz = min(QB, left)
                    batches.append((t, sz))
                    t += sz
                    left -= sz

            def z_mms(t0, m, ps):
                for u in range(m):
                    n0 = (t0 + u) * P128
                    nc.tensor.matmul(
                        out=ps[:, u * H:(u + 1) * H],
                        lhsT=dt[:, n0:n0 + P128], rhs=selw_t[:],
                        start=True, stop=False,
                    )
                    nc.tensor.matmul(
                        out=ps[:, u * H:(u + 1) * H],
                        lhsT=aug_t[:, n0:n0 + P128], rhs=w1_t[:],
                        start=False, stop=True,
                    )

            def g_mms(t0, m, h1):
                for u in range(m):
                    t = t0 + u
                    nc.tensor.matmul(
                        out=G_ps[:],
                        lhsT=h1[:, u * H:(u + 1) * H],
                        rhs=papt_t[:, t * PCOL:(t + 1) * PCOL],
                        start=(t == 0), stop=(t == NT - 1),
                    )

            prev = None
            for bi, (t0, m) in enumerate(batches):
                ps = pszp.tile([P128, QB * H], f32, tag="z")
                z_mms(t0, m, ps)
                h1 = hbuf.tile([P128, QB * H], bf16, tag="h1")
                nc.scalar.activation(
                    out=h1[:, :m * H], in_=ps[:, :m * H],
                    func=mybir.ActivationFunctionType.Relu,
                )
                if prev is not None:
                    g_mms(*prev)
                prev = (t0, m, h1)
            g_mms(*prev)

            G_sb = small.tile([P128, PCOL], f32)
            nc.vector.tensor_copy(out=G_sb[:], in_=G_ps[:])
            nc.sync.dma_start(out=gout[:], in_=G_sb[:])

    nc.compile()
    return nc


def _preprocess(x, edge_index, batch_idx):
    """Integer/structure preprocessing -> per-core device inputs."""
    src = np.asarray(edge_index[0], dtype=np.int64)
    dst = np.asarray(edge_index[1], dtype=np.int64)

    deg = (np.bincount(dst, minlength=N) + 1).astype(np.float32)
    dis = (1.0 / np.sqrt(deg)).astype(np.float32)
    sq = np.sqrt(deg).astype(np.float32)
    dis_pad = np.zeros(NPAD, np.float32)
    dis_pad[:N] = dis
    sq_pad = np.zeros(NPAD, np.float32)
    sq_pad[:N] = sq

    bi = np.asarray(batch_idx, dtype=np.int64)
    cnt = np.bincount(bi, minlength=B).astype(np.float32)

    x_np = np.asarray(x, dtype=np.float32)
    x_pad = np.zeros((NPAD, IN), np.float32)
    x_pad[:N] = x_np
    disx = x_pad * dis_pad[:, None]          # [NPAD, 6]

    # ---- pooling matrices (dense PA = P @ A) ----
    loop = np.arange(N, dtype=np.int64)
    src2 = np.concatenate([src, loop])
    dst2 = np.concatenate([dst, loop])
    w = (dis[src2] * dis[dst2]).astype(np.float64)
    flat = bi[dst2] * NPAD + src2
    PA = np.bincount(flat, weights=w, minlength=B * NPAD).reshape(B, NPAD)
    PA = PA.astype(np.float32)
    Pm = np.zeros((B, NPAD), np.float32)
    Pm[bi, np.arange(N)] = 1.0
    papt_full = (np.concatenate([PA, Pm], axis=0) * dis_pad[None, :]).T  # [NPAD,128]

    # graph span per core (for the P columns)
    first_graph = np.zeros(NG, np.int64)
    span = np.zeros(NG, np.int64)
    for k in range(NG):
        lo, hi = k * NS, min((k + 1) * NS, N)
        if lo >= N:
            first_graph[k] = B - 1
            span[k] = 1
            continue
        gset = bi[lo:hi]
        first_graph[k] = gset[0]
        span[k] = gset[-1] - gset[0] + 1
        assert span[k] <= PCOL - B, f"graph span {span[k]} > {PCOL - B}"

    # ---- per (core, bank) compacted streams (real edges only; the self
    # loop + b1*sqrt(deg) term goes through the aug matmul instead) ----
    core = dst // NS
    bank = src // NS
    src_local = src - bank * NS
    dst_local = dst - core * NS
    chunk2 = np.searchsorted(np.asarray(DOFF[1:]), dst_local, side="right")
    key = ((core * NG + bank) * C + chunk2) * NS + dst_local
    order = np.argsort(key, kind="stable")
    srcl_s = src_local[order]
    dstl_s = dst_local[order]

    cell = ((core * NG + bank) * C + chunk2)[order]
    cellcnt = np.bincount(cell, minlength=NG * NG * C)
    cell_starts = np.zeros(NG * NG * C + 1, np.int64)
    np.cumsum(cellcnt, out=cell_starts[1:])

    # compact column maps per (core, bank)
    colmaps = {}
    ncols = np.zeros((NG, NG), np.int64)
    for k in range(NG):
        for g in range(NG):
            s0 = cell_starts[(k * NG + g) * C]
            s1 = cell_starts[(k * NG + g + 1) * C]
            uniq = np.unique(srcl_s[s0:s1])
            colmaps[(k, g)] = uniq
            ncols[k, g] = len(uniq)
    TW = int(ncols.max()) + 1
    TW = (TW + 15) & ~15

    # per-chunk stream widths (shared across cores for one compiled NEFF)
    cc = cellcnt.reshape(NG, NG, C)
    JWS = []
    for c in range(C):
        m = int(cc[:, :, c].max())
        JWS.append(((m + 1 + 15) // 16) * 16)
    JWT = sum(JWS)
    OFF = np.concatenate([[0], np.cumsum(JWS)]).astype(int)

    # build tables / idx arrays per core
    xt_all = np.zeros((NG, P128, TW), ml_dtypes.bfloat16)
    gidx_all = np.zeros((NG, P128, JWT // 16), np.int16)
    bidx_all = np.zeros((NG, P128, NBT // 16), np.int16)

    for k in range(NG):
        for g in range(NG):
            uniq = colmaps[(k, g)]
            n0 = g * NS
            xt_all[k, 16 * g:16 * g + 6, 1:1 + len(uniq)] = (
                disx[n0 + uniq].T.astype(ml_dtypes.bfloat16)
            )
            # remap this (core, bank)'s stream to compact cols
            s0 = cell_starts[(k * NG + g) * C]
            s1 = cell_starts[(k * NG + g + 1) * C]
            comp = np.searchsorted(uniq, srcl_s[s0:s1]) + 1

            for c in range(C):
                c0 = cell_starts[(k * NG + g) * C + c]
                c1 = cell_starts[(k * NG + g) * C + c + 1]
                ncell = c1 - c0
                stream = np.zeros(JWS[c], np.int64)
                stream[1:1 + ncell] = comp[c0 - s0:c1 - s0]
                blk = stream.reshape(JWS[c] // 16, 16).T.astype(np.int16)
                gidx_all[k, 16 * g:16 * (g + 1), OFF[c] // 16:OFF[c + 1] // 16] = blk

                nd, nb = NDCS[c], NBCS[c]
                dloc = dstl_s[c0:c1] - DOFF[c]
                cnts = np.bincount(dloc, minlength=nd)
                blist = np.zeros(nb, np.int64)
                np.cumsum(cnts, out=blist[1:1 + nd])
                blist[1 + nd:] = blist[nd]
                bblk = blist.reshape(nb // 16, 16).T.astype(np.int16)
                bidx_all[k, 16 * g:16 * (g + 1),
                         BOFF[c] // 16:BOFF[c + 1] // 16] = bblk

    # aug rows: 0-5 dis*x own chunk (self loop), 6 sqrt(deg) (carries b1)
    aug_all = np.zeros((NG, 7, NS), ml_dtypes.bfloat16)
    for k in range(NG):
        n0 = k * NS
        aug_all[k, 0:6] = disx[n0:n0 + NS].T.astype(ml_dtypes.bfloat16)
        aug_all[k, 6] = sq_pad[n0:n0 + NS].astype(ml_dtypes.bfloat16)

    # papt per core: 64 PA cols + local P cols, blocked [NT*128, PCOL]
    papt_all = np.zeros((NG, NT * P128, PCOL), ml_dtypes.bfloat16)
    for k in range(NG):
        n0 = k * NS
        pk = np.zeros((NS, PCOL), np.float32)
        pk[:, :B] = papt_full[n0:n0 + NS, :B]
        b0, sp = first_graph[k], span[k]
        pk[:, B:B + sp] = papt_full[n0:n0 + NS, B + b0:B + b0 + sp]
        papt_all[k] = pk.astype(ml_dtypes.bfloat16)

    return {
        "JW": (TW, tuple(JWS)),
        "TW": TW,
        "JWS": JWS,
        "xt_all": xt_all,
        "gidx_all": gidx_all,
        "bidx_all": bidx_all,
        "aug_all": aug_all,
        "papt_all": papt_all,
        "first_graph": first_graph,
        "span": span,
        "cnt": cnt,
    }


def _head(G, cnt, inputs):
    f = np.float32
    W2 = np.asarray(inputs["W2"], f)
    b2 = np.asarray(inputs["b2"], f)
    Wg = np.asarray(inputs["Wg"], f)
    bg = np.asarray(inputs["bg"], f)
    Et = np.asarray(inputs["Et"], f)
    Ek = np.asarray(inputs["Ek"], f)
    Ev = np.asarray(inputs["Ev"], f)
    Wp = np.asarray(inputs["Wp"], f)
    bp = np.asarray(inputs["bp"], f)
    Ekid = np.asarray(inputs["Ekid"], f)
    Wc = np.asarray(inputs["Wc"], f)
    bc = np.asarray(inputs["bc"], f)
    Wl = np.asarray(inputs["Wl"], f)
    bl = np.asarray(inputs["bl"], f)
    Wm1 = np.asarray(inputs["Wm1"], f)
    bm1 = np.asarray(inputs["bm1"], f)
    Wm2 = np.asarray(inputs["Wm2"], f)
    bm2 = np.asarray(inputs["bm2"], f)
    st = np.asarray(inputs["sol_type_idx"], np.int64)
    sk = np.asarray(inputs["sol_key_idx"], np.int64)
    sv = np.asarray(inputs["sol_val_idx"], np.int64)
    kid = np.asarray(inputs["kernel_id"], np.int64)
    cond = np.asarray(inputs["cond_vec"], f)
    loc = np.asarray(inputs["local_feats"], f)

    relu = lambda a: np.maximum(a, 0.0).astype(f)

    Ph2 = G[:B] @ W2 + cnt[:, None] * b2[None, :] + G[B:]
    g = (Ph2 / np.maximum(cnt, 1.0)[:, None]) @ Wg + bg

    seq_mean = np.concatenate(
        [Et[st].mean(axis=1), Ek[sk].mean(axis=1), Ev[sv].mean(axis=1)], axis=-1
    ).astype(f)
    p = relu(seq_mean @ Wp + bp)
    kvec = Ekid[kid]
    c = relu(cond @ Wc + bc)
    l = relu(loc @ Wl + bl)
    xf = np.concatenate([g, p, kvec, c, l], axis=1).astype(f)
    return (relu(xf @ Wm1 + bm1) @ Wm2 + bm2).astype(f)


def kernel(**inputs) -> np.ndarray:
    from concourse.bass_utils import run_bass_kernel_spmd

    pre = _preprocess(inputs["x"], inputs["edge_index"], inputs["batch_idx"])
    sig = pre["JW"]
    if sig not in _compiled:
        _compiled[sig] = _build_nc(pre["TW"], tuple(pre["JWS"]))
    nc = _compiled[sig]

    W1 = np.asarray(inputs["W1"], np.float32)
    b1 = np.asarray(inputs["b1"], np.float32)
    w1aug = np.concatenate([W1, b1[None, :]], axis=0).astype(ml_dtypes.bfloat16)
    selw = np.zeros((P128, H), ml_dtypes.bfloat16)
    for g in range(NG):
        selw[16 * g:16 * g + 6] = W1.astype(ml_dtypes.bfloat16)

    in_maps = []
    for k in range(NG):
        in_maps.append({
            "xt": pre["xt_all"][k],
            "gidx": pre["gidx_all"][k],
            "bidx": pre["bidx_all"][k],
            "aug": pre["aug_all"][k],
            "selw": selw,
            "w1aug": w1aug,
            "papt": pre["papt_all"][k],
        })

    res = run_bass_kernel_spmd(nc, in_maps, core_ids=list(range(NG)))

    Gpa = np.zeros((B, H), np.float64)
    Gp = np.zeros((B, H), np.float64)
    for k, r in enumerate(res.results):
        gt = r["gout"].astype(np.float64)      # [128 f, 80 c]
        Gpa += gt[:, :B].T
        b0, sp = pre["first_graph"][k], pre["span"][k]
        Gp[b0:b0 + sp] += gt[:, B:B + sp].T
    G = np.concatenate([Gpa, Gp], axis=0).astype(np.float32)   # [128, H]

    return _head(G, pre["cnt"], inputs)
